# revision 27
# baseline (speedup 1.0000x reference)
"""Multi-head attention (B=2, S=2048, H=2048, NH=16) on 8 TRN2 NeuronCores.

Sharding: tensor-parallel over heads — 2 heads per core. Each core computes
q/k/v projections for its heads, per-head attention, and a partial output
projection (its heads' columns of Wo); the host sums the 8 partials.

v3: software-pipelined emission. Attention is ScalarE-bound (exp); qkv and
oproj matmul units are emitted as "fillers" between attention chunks so the
in-order TensorE queue always has ready work during exp waits.
  - PSUM: "sc" [128,1024] x2 (scores, 4 banks), "av" [128,512] x2 (AV
    accumulators, 2 banks), "half" [128,512] x2 (den / qkv chunks / filler
    oproj / fast oproj, 2 banks).
  - softmax denominator: shallow DVE add tree (depth ~5) to one [128,1024]
    bf16 sum + single deferred ones-matmul per (b,h,qh) group, flushed at
    the NEXT group's start (before its ps_av allocations — ordering matters:
    TensorE is in-order, so the den matmul must precede any matmul that
    waits on the slots it frees).
  - oproj fast path: 2 token-tiles in flight (sc slots / av+half slots);
    drains split between ScalarE ([128,1024] copies) and VectorE.
  - output rows DMA'd as [128,2048] tiles (4KB lines) on sync/gpsimd;
    hT streamed in 4-fc-chunk DMAs (sync for b=0, gpsimd for b=1);
    weights 2 chunks each, earliest-consumed first.
"""

import sys

sys.path.insert(0, "/opt/trn_rl_repo")

from contextlib import ExitStack

import ml_dtypes
import numpy as np

import concourse.bass as bass
import concourse.tile as tile
from concourse import bacc, mybir
from concourse.bass_utils import run_bass_kernel_spmd

B, S, H, NH = 2, 2048, 2048, 16
HD = H // NH          # 128
N_CORES = 8
HPC = NH // N_CORES   # heads per core = 2
HDC = HPC * HD        # head-dims per core = 256
T = B * S             # 4096 tokens
FC = H // 128         # 16 feature chunks
TC = S // 128         # 16 token tiles per batch
SHIFT = 4.0           # fixed exp shift (softmax-invariant, overflow guard)

BF16 = mybir.dt.bfloat16
F32 = mybir.dt.float32
EXP = mybir.ActivationFunctionType.Exp
COPY = mybir.ActivationFunctionType.Copy

_CACHE = {}


def build_program(out_dtype=BF16):
    nc = bacc.Bacc(
        "TRN2", target_bir_lowering=False, debug=False, num_devices=N_CORES
    )
    # all pre-tiled host-side to partition-major so DMA lines are 4-16KB
    hT = nc.dram_tensor("hT", [8 * 128, FC, 512], BF16, kind="ExternalInput").ap()
    wqT = nc.dram_tensor("wqT", [128, FC, HDC], BF16, kind="ExternalInput").ap()
    wkT = nc.dram_tensor("wkT", [128, FC, HDC], BF16, kind="ExternalInput").ap()
    wvT = nc.dram_tensor("wvT", [128, FC, HDC], BF16, kind="ExternalInput").ap()
    woT = nc.dram_tensor("woT", [128, HPC, H], BF16, kind="ExternalInput").ap()
    bq = nc.dram_tensor("bq", [HDC], F32, kind="ExternalInput").ap()
    bk = nc.dram_tensor("bk", [HDC], F32, kind="ExternalInput").ap()
    bv = nc.dram_tensor("bv", [1, HDC], F32, kind="ExternalInput").ap()
    out = nc.dram_tensor("out", [T, H], out_dtype, kind="ExternalOutput").ap()

    with tile.TileContext(nc) as tc:
        _kernel(tc, out, hT, wqT, wkT, wvT, woT, bq, bk, bv)
    nc.compile()
    return nc


def _kernel(tc, out, hT, wqT, wkT, wvT, woT, bq, bk, bv):
    nc = tc.nc
    scale = 1.0 / float(np.sqrt(HD))
    ctx = ExitStack()
    with ctx:
        singles = ctx.enter_context(tc.tile_pool(name="singles", bufs=1))
        persist = ctx.enter_context(tc.tile_pool(name="persist", bufs=1))
        ps_sc = ctx.enter_context(tc.tile_pool(name="ps_sc", bufs=2, space="PSUM"))
        ps_av = ctx.enter_context(tc.tile_pool(name="ps_av", bufs=2, space="PSUM"))
        ps_half = ctx.enter_context(tc.tile_pool(name="ps_half", bufs=2, space="PSUM"))
        ht_pool = ctx.enter_context(tc.tile_pool(name="ht", bufs=3))
        pt_pool = ctx.enter_context(tc.tile_pool(name="pt", bufs=5))
        ts_pool = ctx.enter_context(tc.tile_pool(name="ts", bufs=6))
        rc_pool = ctx.enter_context(tc.tile_pool(name="rc", bufs=2))
        o_pool = ctx.enter_context(tc.tile_pool(name="o_sb", bufs=4))

        # ---- constants ----
        ones = singles.tile([128, 128], BF16)
        nc.vector.memset(ones, 1.0)
        neg_shift = singles.tile([128, 1], F32)
        nc.vector.memset(neg_shift, -SHIFT)

        # ---- weights: 2-chunk DMAs, earliest-consumed first ----
        w_sb = {}
        for name in ("v", "q", "k"):
            w_sb[name] = singles.tile(
                [128, FC, HDC], BF16, tag=f"w{name}", name=f"w{name}"
            )
        for g in range(4):
            nc.gpsimd.dma_start(
                out=w_sb["v"][:, 4 * g : 4 * g + 4, :],
                in_=wvT[:, 4 * g : 4 * g + 4, :],
            )
        for name, ap in (("q", wqT), ("k", wkT)):
            for g in range(2):
                nc.gpsimd.dma_start(
                    out=w_sb[name][:, 8 * g : 8 * g + 8, :],
                    in_=ap[:, 8 * g : 8 * g + 8, :],
                )
        woT_sb = singles.tile([128, HPC, H], BF16)
        nc.gpsimd.dma_start(out=woT_sb, in_=woT)
        bq_sb = singles.tile([128, HPC], F32)
        nc.scalar.dma_start(out=bq_sb, in_=bq.rearrange("(h p) -> p h", p=128))
        bk_sb = singles.tile([128, HPC], F32)
        nc.scalar.dma_start(out=bk_sb, in_=bk.rearrange("(h p) -> p h", p=128))
        bv2 = singles.tile([128, 2, HDC], F32)
        nc.scalar.dma_start(
            out=bv2,
            in_=bass.AP(tensor=bv.tensor, offset=bv.offset,
                        ap=[[0, 128], [0, 2], [1, HDC]]),
        )

        # ---- persistent activations ----
        qt_sb = [[persist.tile([128, S], BF16, tag=f"qt{b}{h}", name=f"qt{b}{h}")
                  for h in range(HPC)] for b in range(B)]
        kt_sb = [[persist.tile([128, S], BF16, tag=f"kt{b}{h}", name=f"kt{b}{h}")
                  for h in range(HPC)] for b in range(B)]
        v_sb = [persist.tile([128, TC, HDC], BF16, tag=f"v{b}", name=f"v{b}")
                for b in range(B)]
        aoT_sb = [[persist.tile([128, S], BF16, tag=f"ao{b}{h}", name=f"ao{b}{h}")
                   for h in range(HPC)] for b in range(B)]

        ht_tiles = {}

        def load_ht(b, blk):
            key = (b, blk)
            if key in ht_tiles:
                return ht_tiles[key]
            bi = b * 4 + blk
            t = ht_pool.tile([128, FC, 512], BF16, tag="ht", name=f"ht{b}{blk}")
            eng = nc.sync if b == 0 else nc.gpsimd
            for g in range(4):
                eng.dma_start(
                    out=t[:, 4 * g : 4 * g + 4, :],
                    in_=hT[bi * 128 : (bi + 1) * 128, 4 * g : 4 * g + 4, :],
                )
            ht_tiles[key] = t
            return t

        # ---- deferred softmax denominator ----
        pending = []

        def flush_pending():
            while pending:
                pending.pop(0)()

        # ---- filler queue (units of TensorE work to hide exp waits) ----
        filler_q = []

        def fill(n):
            while n > 0 and filler_q:
                try:
                    next(filler_q[0])
                    n -= 1
                except StopIteration:
                    filler_q.pop(0)

        def drain_fillers():
            while filler_q:
                fill(1)

        # ---- qkv projection units ----
        def v_block(b, ht_t, blk, s2):
            ps = ps_half.tile([128, 2, HDC], F32, tag="half",
                              name=f"v{b}{blk}{s2}")
            for s in range(2):
                col = (2 * s2 + s) * 128
                for fc in range(FC):
                    nc.tensor.matmul(
                        ps[:, s, :],
                        ht_t[:, fc, col : col + 128],
                        w_sb["v"][:, fc, :],
                        start=(fc == 0),
                        stop=(fc == FC - 1),
                    )
            tt0 = blk * 4 + s2 * 2
            nc.vector.tensor_add(v_sb[b][:, tt0 : tt0 + 2, :], ps, bv2)

        def qk_block(b, ht_t, blk, h, name, dst, bias):
            ps = ps_half.tile([128, 512], F32, tag="half",
                              name=f"qk{b}{blk}{h}{name}")
            for fc in range(FC):
                nc.tensor.matmul(
                    ps,
                    w_sb[name][:, fc, h * HD : (h + 1) * HD],
                    ht_t[:, fc, :],
                    start=(fc == 0),
                    stop=(fc == FC - 1),
                )
            nc.vector.tensor_scalar_add(
                dst[:, blk * 512 : (blk + 1) * 512], ps, bias[:, h : h + 1],
            )

        def qkv_gen(b, half):
            for qx in range(2):
                blk = half * 2 + qx
                ht_t = load_ht(b, blk)
                if blk + 1 < 4:
                    load_ht(b, blk + 1)
                elif b == 0:
                    load_ht(1, 0)
                v_block(b, ht_t, blk, 0)
                yield
                for h in range(HPC):
                    qk_block(b, ht_t, blk, h, "q", qt_sb[b][h], bq_sb)
                    yield
                    qk_block(b, ht_t, blk, h, "k", kt_sb[b][h], bk_sb)
                    yield
                v_block(b, ht_t, blk, 1)
                yield

        def qkv_half(b, half):
            for _ in qkv_gen(b, half):
                pass

        # ---- attention ----
        def attention(b, qh, fill_every=2):
            q0 = qh * 1024
            for h in range(HPC):
                av = [ps_av.tile([128, 512], F32, tag="av",
                                 name=f"av{b}{h}{qh}{n}") for n in range(2)]
                pt_prev = None
                t2_prev = None
                acc = None
                for tcx in range(TC):
                    ps = ps_sc.tile([128, 1024], F32, tag="sc",
                                    name=f"sc{b}{h}{qh}{tcx}")
                    lhsT = kt_sb[b][h][:, tcx * 128 : (tcx + 1) * 128]
                    for n in range(2):
                        nc.tensor.matmul(
                            ps[:, n * 512 : (n + 1) * 512],
                            lhsT,
                            qt_sb[b][h][:, q0 + n * 512 : q0 + (n + 1) * 512],
                            start=True,
                            stop=True,
                        )
                    pt = pt_pool.tile([128, 1024], BF16, tag="pt",
                                      name=f"pt{b}{h}{qh}{tcx}")
                    nc.scalar.activation(pt, ps, EXP,
                                         bias=neg_shift, scale=scale)
                    if tcx == 0:
                        # previous group's den/normalize: emitted after this
                        # group's first scores+exp (covers the DVE-chain
                        # latency) but before the first av matmul reuses
                        # the ps_av slots it still holds.
                        flush_pending()
                    for n in range(2):
                        nc.tensor.matmul(
                            av[n],
                            v_sb[b][:, tcx, h * HD : (h + 1) * HD],
                            pt[:, n * 512 : (n + 1) * 512],
                            start=(tcx == 0),
                            stop=(tcx == TC - 1),
                        )
                    if tcx % fill_every == fill_every - 1:
                        fill(1)
                    # denominator tree: pt pairs -> t2, t2 pairs -> p4,
                    # chain p4s (depth ~5 in bf16)
                    if pt_prev is None:
                        pt_prev = pt
                    else:
                        t2 = ts_pool.tile([128, 1024], BF16, tag="ts",
                                          name=f"t2{b}{h}{qh}{tcx}")
                        nc.vector.tensor_add(t2, pt_prev, pt)
                        pt_prev = None
                        if t2_prev is None:
                            t2_prev = t2
                        else:
                            p4 = ts_pool.tile([128, 1024], BF16, tag="ts",
                                              name=f"p4{b}{h}{qh}{tcx}")
                            nc.vector.tensor_add(p4, t2_prev, t2)
                            t2_prev = None
                            if acc is None:
                                acc = p4
                            else:
                                nacc = ts_pool.tile(
                                    [128, 1024], BF16, tag="ts",
                                    name=f"acc{b}{h}{qh}{tcx}")
                                nc.vector.tensor_add(nacc, acc, p4)
                                acc = nacc

                def den_norm(b=b, h=h, q0=q0, av=av, acc=acc):
                    rc = rc_pool.tile([128, 1024], F32, tag="rc",
                                      name=f"rc{b}{h}{q0}")
                    for n in range(2):
                        den = ps_half.tile([128, 512], F32, tag="half",
                                           name=f"den{b}{h}{q0}{n}")
                        nc.tensor.matmul(
                            den, ones, acc[:, n * 512 : (n + 1) * 512],
                            start=True, stop=True,
                        )
                        nc.vector.reciprocal_approx_fast(
                            rc[:, n * 512 : (n + 1) * 512], den)
                        nc.vector.tensor_mul(
                            aoT_sb[b][h][:, q0 + n * 512 : q0 + (n + 1) * 512],
                            av[n], rc[:, n * 512 : (n + 1) * 512])

                pending.append(den_norm)

        # ---- output projection ----
        def oproj_mm(b, tt, pss):
            # pss: 4 (psum_tile, col0) pairs covering [128, 2048]
            for h in range(HPC):
                lhsT = aoT_sb[b][h][:, tt * 128 : (tt + 1) * 128]
                for ps, oc0, w in pss:
                    nc.tensor.matmul(
                        ps,
                        lhsT,
                        woT_sb[:, h, oc0 : oc0 + w],
                        start=(h == 0),
                        stop=(h == HPC - 1),
                    )

        def oproj_gen(b, tts):
            # attention-safe filler units: only ps_half slots, DVE drains
            for tt in tts:
                o_t = o_pool.tile([128, H], BF16, tag="o", name=f"ot{b}{tt}")
                for ocp in range(2):
                    for oc in (2 * ocp, 2 * ocp + 1):
                        ps = ps_half.tile([128, 512], F32, tag="half",
                                          name=f"op{b}{tt}{oc}")
                        for h in range(HPC):
                            nc.tensor.matmul(
                                ps,
                                aoT_sb[b][h][:, tt * 128 : (tt + 1) * 128],
                                woT_sb[:, h, oc * 512 : (oc + 1) * 512],
                                start=(h == 0),
                                stop=(h == HPC - 1),
                            )
                        nc.vector.tensor_copy(
                            o_t[:, oc * 512 : (oc + 1) * 512], ps)
                    if ocp == 1:
                        row0 = b * S + tt * 128
                        eng = nc.sync if tt % 2 == 0 else nc.gpsimd
                        eng.dma_start(out=out[row0 : row0 + 128, :], in_=o_t)
                    yield

        def oproj_fast(b, tts):
            # post-attention path: 2 token-tiles in flight, drains split
            # between ScalarE (sc-slot tiles) and VectorE (av+half tiles)
            for tt in tts:
                o_t = o_pool.tile([128, H], BF16, tag="o", name=f"ot{b}{tt}")
                if tt % 2 == 0:
                    tiles = [ps_sc.tile([128, 1024], F32, tag="sc",
                                        name=f"op{b}{tt}{k}") for k in range(2)]
                    pss = []
                    for k in range(2):
                        pss.append((tiles[k][:, 0:512], k * 1024, 512))
                        pss.append((tiles[k][:, 512:1024], k * 1024 + 512, 512))
                    oproj_mm(b, tt, pss)
                    for k in range(2):
                        nc.scalar.activation(
                            o_t[:, k * 1024 : (k + 1) * 1024], tiles[k], COPY)
                else:
                    pss = []
                    for oc in range(4):
                        pool, tag = (ps_av, "av") if oc >= 2 else (ps_half, "half")
                        ps = pool.tile([128, 512], F32, tag=tag,
                                       name=f"op{b}{tt}{oc}")
                        pss.append((ps, oc * 512, 512))
                    oproj_mm(b, tt, pss)
                    for oc in range(4):
                        nc.vector.tensor_copy(
                            o_t[:, oc * 512 : (oc + 1) * 512], pss[oc][0])
                row0 = b * S + tt * 128
                eng = nc.sync if tt % 2 == 0 else nc.gpsimd
                eng.dma_start(out=out[row0 : row0 + 128, :], in_=o_t)

        # ---- main schedule ----
        qkv_half(0, 0)
        qkv_half(0, 1)
        filler_q.append(qkv_gen(1, 0))
        attention(0, 0)
        filler_q.append(qkv_gen(1, 1))
        filler_q.append(oproj_gen(0, list(range(0, 8))))
        attention(0, 1)
        drain_fillers()          # qkv(1,*) must complete before att(1,*)
        filler_q.append(oproj_gen(0, list(range(8, TC))))
        attention(1, 0, fill_every=3)
        filler_q.append(oproj_gen(1, list(range(0, 8))))
        attention(1, 1, fill_every=3)
        drain_fillers()
        flush_pending()
        oproj_fast(1, list(range(8, TC)))
        flush_pending()


def kernel(hidden_state, Wq, bq, Wk, bk, Wv, bv, Wo, bo):
    bf16 = ml_dtypes.bfloat16
    h2 = np.asarray(hidden_state, dtype=np.float32).reshape(T, H)
    hT = np.ascontiguousarray(h2.T).astype(bf16)  # [H, T], H = (c p)
    # pre-tile to [ (b blk p), c, t ] so each tile DMA is contiguous
    hT_t = np.ascontiguousarray(
        hT.reshape(FC, 128, B, 4, 512).transpose(2, 3, 1, 0, 4)
    ).reshape(8 * 128, FC, 512)

    def tile_w(w_slice_T):  # [H, HDC] -> [128, FC, HDC] partition-major
        return np.ascontiguousarray(
            w_slice_T.reshape(FC, 128, HDC).transpose(1, 0, 2)).astype(bf16)

    in_maps = []
    for c in range(N_CORES):
        r0 = c * HDC
        woT = np.asarray(Wo, np.float32)[:, r0 : r0 + HDC].T  # [HDC, H]
        in_maps.append({
            "hT": hT_t,
            "wqT": tile_w(np.asarray(Wq, np.float32)[r0 : r0 + HDC, :].T),
            "wkT": tile_w(np.asarray(Wk, np.float32)[r0 : r0 + HDC, :].T),
            "wvT": tile_w(np.asarray(Wv, np.float32)[r0 : r0 + HDC, :].T),
            "woT": np.ascontiguousarray(
                woT.reshape(HPC, 128, H).transpose(1, 0, 2)).astype(bf16),
            "bq": np.asarray(bq, np.float32)[r0 : r0 + HDC].copy(),
            "bk": np.asarray(bk, np.float32)[r0 : r0 + HDC].copy(),
            "bv": np.asarray(bv, np.float32)[r0 : r0 + HDC].reshape(1, HDC).copy(),
        })

    if "nc" not in _CACHE:
        _CACHE["nc"] = build_program()
    nc = _CACHE["nc"]
    _CACHE["in_maps"] = in_maps

    res = run_bass_kernel_spmd(nc, in_maps, core_ids=list(range(N_CORES)))
    total = np.zeros((T, H), np.float32)
    for r in res.results:
        total += np.asarray(r["out"]).astype(np.float32)
    total += np.asarray(bo, np.float32)[None, :]
    return total.reshape(B, S, H)


# revision 30
# speedup vs baseline: 1.0234x; 1.0234x over previous
"""Multi-head attention (B=2, S=2048, H=2048, NH=16) on 8 TRN2 NeuronCores.

Sharding: tensor-parallel over heads — 2 heads per core. Each core computes
q/k/v projections for its heads, per-head attention, and a partial output
projection (its heads' columns of Wo); the host sums the 8 partials.

v3: software-pipelined emission. Attention is ScalarE-bound (exp); qkv and
oproj matmul units are emitted as "fillers" between attention chunks so the
in-order TensorE queue always has ready work during exp waits.
  - PSUM: "sc" [128,1024] x2 (scores, 4 banks), "av" [128,512] x2 (AV
    accumulators, 2 banks), "half" [128,512] x2 (den / qkv chunks / filler
    oproj / fast oproj, 2 banks).
  - softmax denominator: shallow DVE add tree (depth ~5) to one [128,1024]
    bf16 sum + single deferred ones-matmul per (b,h,qh) group, flushed at
    the NEXT group's start (before its ps_av allocations — ordering matters:
    TensorE is in-order, so the den matmul must precede any matmul that
    waits on the slots it frees).
  - oproj fast path: 2 token-tiles in flight (sc slots / av+half slots);
    drains split between ScalarE ([128,1024] copies) and VectorE.
  - output rows DMA'd as [128,2048] tiles (4KB lines) on sync/gpsimd;
    hT streamed in 4-fc-chunk DMAs (sync for b=0, gpsimd for b=1);
    weights 2 chunks each, earliest-consumed first.
"""

import sys

sys.path.insert(0, "/opt/trn_rl_repo")

from contextlib import ExitStack

import ml_dtypes
import numpy as np

import concourse.bass as bass
import concourse.tile as tile
from concourse import bacc, mybir
from concourse.bass_utils import run_bass_kernel_spmd

B, S, H, NH = 2, 2048, 2048, 16
HD = H // NH          # 128
N_CORES = 8
HPC = NH // N_CORES   # heads per core = 2
HDC = HPC * HD        # head-dims per core = 256
T = B * S             # 4096 tokens
FC = H // 128         # 16 feature chunks
TC = S // 128         # 16 token tiles per batch
SHIFT = 4.0           # fixed exp shift (softmax-invariant, overflow guard)

BF16 = mybir.dt.bfloat16
F32 = mybir.dt.float32
EXP = mybir.ActivationFunctionType.Exp
COPY = mybir.ActivationFunctionType.Copy

_CACHE = {}


def build_program(out_dtype=BF16):
    nc = bacc.Bacc(
        "TRN2", target_bir_lowering=False, debug=False, num_devices=N_CORES
    )
    # all pre-tiled host-side to partition-major so DMA lines are 4-16KB
    hT = nc.dram_tensor("hT", [8 * 128, FC, 512], BF16, kind="ExternalInput").ap()
    wqT = nc.dram_tensor("wqT", [128, FC, HDC], BF16, kind="ExternalInput").ap()
    wkT = nc.dram_tensor("wkT", [128, FC, HDC], BF16, kind="ExternalInput").ap()
    wvT = nc.dram_tensor("wvT", [128, FC, HDC], BF16, kind="ExternalInput").ap()
    woT = nc.dram_tensor("woT", [128, HPC, H], BF16, kind="ExternalInput").ap()
    bq = nc.dram_tensor("bq", [HDC], F32, kind="ExternalInput").ap()
    bk = nc.dram_tensor("bk", [HDC], F32, kind="ExternalInput").ap()
    bv = nc.dram_tensor("bv", [1, HDC], F32, kind="ExternalInput").ap()
    out = nc.dram_tensor("out", [T, H], out_dtype, kind="ExternalOutput").ap()

    with tile.TileContext(nc) as tc:
        _kernel(tc, out, hT, wqT, wkT, wvT, woT, bq, bk, bv)
    nc.compile()
    return nc


def _kernel(tc, out, hT, wqT, wkT, wvT, woT, bq, bk, bv):
    nc = tc.nc
    scale = 1.0 / float(np.sqrt(HD))
    ctx = ExitStack()
    with ctx:
        singles = ctx.enter_context(tc.tile_pool(name="singles", bufs=1))
        persist = ctx.enter_context(tc.tile_pool(name="persist", bufs=1))
        ps_sc = ctx.enter_context(tc.tile_pool(name="ps_sc", bufs=2, space="PSUM"))
        ps_av = ctx.enter_context(tc.tile_pool(name="ps_av", bufs=2, space="PSUM"))
        ps_half = ctx.enter_context(tc.tile_pool(name="ps_half", bufs=2, space="PSUM"))
        ht_pool = ctx.enter_context(tc.tile_pool(name="ht", bufs=3))
        pt_pool = ctx.enter_context(tc.tile_pool(name="pt", bufs=5))
        ts_pool = ctx.enter_context(tc.tile_pool(name="ts", bufs=6))
        rc_pool = ctx.enter_context(tc.tile_pool(name="rc", bufs=2))
        o_pool = ctx.enter_context(tc.tile_pool(name="o_sb", bufs=4))

        # ---- constants ----
        ones = singles.tile([128, 128], BF16)
        nc.vector.memset(ones, 1.0)
        neg_shift = singles.tile([128, 1], F32)
        nc.vector.memset(neg_shift, -SHIFT)

        # ---- weights: 2-chunk DMAs, earliest-consumed first ----
        w_sb = {}
        for name in ("v", "q", "k"):
            w_sb[name] = singles.tile(
                [128, FC, HDC], BF16, tag=f"w{name}", name=f"w{name}"
            )
        for g in range(4):
            nc.gpsimd.dma_start(
                out=w_sb["v"][:, 4 * g : 4 * g + 4, :],
                in_=wvT[:, 4 * g : 4 * g + 4, :],
            )
        for name, ap in (("q", wqT), ("k", wkT)):
            for g in range(2):
                nc.gpsimd.dma_start(
                    out=w_sb[name][:, 8 * g : 8 * g + 8, :],
                    in_=ap[:, 8 * g : 8 * g + 8, :],
                )
        woT_sb = singles.tile([128, HPC, H], BF16)
        nc.gpsimd.dma_start(out=woT_sb, in_=woT)
        bq_sb = singles.tile([128, HPC], F32)
        nc.scalar.dma_start(out=bq_sb, in_=bq.rearrange("(h p) -> p h", p=128))
        bk_sb = singles.tile([128, HPC], F32)
        nc.scalar.dma_start(out=bk_sb, in_=bk.rearrange("(h p) -> p h", p=128))
        bv2 = singles.tile([128, 2, HDC], F32)
        nc.scalar.dma_start(
            out=bv2,
            in_=bass.AP(tensor=bv.tensor, offset=bv.offset,
                        ap=[[0, 128], [0, 2], [1, HDC]]),
        )

        # ---- persistent activations ----
        qt_sb = [[persist.tile([128, S], BF16, tag=f"qt{b}{h}", name=f"qt{b}{h}")
                  for h in range(HPC)] for b in range(B)]
        kt_sb = [[persist.tile([128, S], BF16, tag=f"kt{b}{h}", name=f"kt{b}{h}")
                  for h in range(HPC)] for b in range(B)]
        v_sb = [persist.tile([128, TC, HDC], BF16, tag=f"v{b}", name=f"v{b}")
                for b in range(B)]
        aoT_sb = [[persist.tile([128, S], BF16, tag=f"ao{b}{h}", name=f"ao{b}{h}")
                   for h in range(HPC)] for b in range(B)]

        ht_tiles = {}

        def load_ht(b, blk):
            key = (b, blk)
            if key in ht_tiles:
                return ht_tiles[key]
            bi = b * 4 + blk
            t = ht_pool.tile([128, FC, 512], BF16, tag="ht", name=f"ht{b}{blk}")
            eng = nc.sync if b == 0 else nc.gpsimd
            for g in range(4):
                eng.dma_start(
                    out=t[:, 4 * g : 4 * g + 4, :],
                    in_=hT[bi * 128 : (bi + 1) * 128, 4 * g : 4 * g + 4, :],
                )
            ht_tiles[key] = t
            return t

        # ---- deferred softmax denominator ----
        pending = []

        def flush_pending():
            while pending:
                pending.pop(0)()

        # ---- filler queue (units of TensorE work to hide exp waits) ----
        filler_q = []

        def fill(n):
            while n > 0 and filler_q:
                try:
                    next(filler_q[0])
                    n -= 1
                except StopIteration:
                    filler_q.pop(0)

        def drain_fillers():
            while filler_q:
                fill(1)

        # ---- qkv projection units ----
        def v_block(b, ht_t, blk, s2):
            ps = ps_half.tile([128, 2, HDC], F32, tag="half",
                              name=f"v{b}{blk}{s2}")
            for s in range(2):
                col = (2 * s2 + s) * 128
                for fc in range(FC):
                    nc.tensor.matmul(
                        ps[:, s, :],
                        ht_t[:, fc, col : col + 128],
                        w_sb["v"][:, fc, :],
                        start=(fc == 0),
                        stop=(fc == FC - 1),
                    )
            tt0 = blk * 4 + s2 * 2
            nc.vector.tensor_add(v_sb[b][:, tt0 : tt0 + 2, :], ps, bv2)

        def qk_block(b, ht_t, blk, h, name, dst, bias):
            ps = ps_half.tile([128, 512], F32, tag="half",
                              name=f"qk{b}{blk}{h}{name}")
            for fc in range(FC):
                nc.tensor.matmul(
                    ps,
                    w_sb[name][:, fc, h * HD : (h + 1) * HD],
                    ht_t[:, fc, :],
                    start=(fc == 0),
                    stop=(fc == FC - 1),
                )
            nc.vector.tensor_scalar_add(
                dst[:, blk * 512 : (blk + 1) * 512], ps, bias[:, h : h + 1],
            )

        def qkv_gen(b, half):
            for qx in range(2):
                blk = half * 2 + qx
                ht_t = load_ht(b, blk)
                if blk + 1 < 4:
                    load_ht(b, blk + 1)
                elif b == 0:
                    load_ht(1, 0)
                v_block(b, ht_t, blk, 0)
                yield
                for h in range(HPC):
                    qk_block(b, ht_t, blk, h, "q", qt_sb[b][h], bq_sb)
                    yield
                    qk_block(b, ht_t, blk, h, "k", kt_sb[b][h], bk_sb)
                    yield
                v_block(b, ht_t, blk, 1)
                yield

        def qkv_half(b, half):
            for _ in qkv_gen(b, half):
                pass

        # ---- attention ----
        def attention(b, qh, fill_every=2):
            q0 = qh * 1024
            for h in range(HPC):
                av = [ps_av.tile([128, 512], F32, tag="av",
                                 name=f"av{b}{h}{qh}{n}") for n in range(2)]
                st = {"pt_prev": None, "t2_prev": None, "acc": None}

                def post(j, pt, b=b, h=h, qh=qh, av=av, st=st,
                         fill_every=fill_every):
                    # av matmuls + denominator-tree step for chunk j,
                    # emitted one chunk late so they never wait on exp
                    for n in range(2):
                        nc.tensor.matmul(
                            av[n],
                            v_sb[b][:, j, h * HD : (h + 1) * HD],
                            pt[:, n * 512 : (n + 1) * 512],
                            start=(j == 0),
                            stop=(j == TC - 1),
                        )
                    if j % fill_every == fill_every - 1:
                        fill(1)
                    # tree: pt pairs -> t2, t2 pairs -> p4, chain p4s
                    if st["pt_prev"] is None:
                        st["pt_prev"] = pt
                    else:
                        t2 = ts_pool.tile([128, 1024], BF16, tag="ts",
                                          name=f"t2{b}{h}{qh}{j}")
                        nc.vector.tensor_add(t2, st["pt_prev"], pt)
                        st["pt_prev"] = None
                        if st["t2_prev"] is None:
                            st["t2_prev"] = t2
                        else:
                            p4 = ts_pool.tile([128, 1024], BF16, tag="ts",
                                              name=f"p4{b}{h}{qh}{j}")
                            nc.vector.tensor_add(p4, st["t2_prev"], t2)
                            st["t2_prev"] = None
                            if st["acc"] is None:
                                st["acc"] = p4
                            else:
                                nacc = ts_pool.tile(
                                    [128, 1024], BF16, tag="ts",
                                    name=f"acc{b}{h}{qh}{j}")
                                nc.vector.tensor_add(nacc, st["acc"], p4)
                                st["acc"] = nacc

                prev = None  # (j, pt) of the previous chunk
                for tcx in range(TC):
                    ps = ps_sc.tile([128, 1024], F32, tag="sc",
                                    name=f"sc{b}{h}{qh}{tcx}")
                    lhsT = kt_sb[b][h][:, tcx * 128 : (tcx + 1) * 128]
                    for n in range(2):
                        nc.tensor.matmul(
                            ps[:, n * 512 : (n + 1) * 512],
                            lhsT,
                            qt_sb[b][h][:, q0 + n * 512 : q0 + (n + 1) * 512],
                            start=True,
                            stop=True,
                        )
                    pt = pt_pool.tile([128, 1024], BF16, tag="pt",
                                      name=f"pt{b}{h}{qh}{tcx}")
                    nc.scalar.activation(pt, ps, EXP,
                                         bias=neg_shift, scale=scale)
                    if tcx == 0:
                        # previous group's den/normalize: emitted after this
                        # group's first scores+exp (covers the DVE-chain
                        # latency) but before the first av matmul reuses
                        # the ps_av slots it still holds.
                        flush_pending()
                    if prev is not None:
                        post(*prev)
                    prev = (tcx, pt)
                post(*prev)
                acc = st["acc"]

                def den_norm(b=b, h=h, q0=q0, av=av, acc=acc):
                    rc = rc_pool.tile([128, 1024], F32, tag="rc",
                                      name=f"rc{b}{h}{q0}")
                    for n in range(2):
                        den = ps_half.tile([128, 512], F32, tag="half",
                                           name=f"den{b}{h}{q0}{n}")
                        nc.tensor.matmul(
                            den, ones, acc[:, n * 512 : (n + 1) * 512],
                            start=True, stop=True,
                        )
                        nc.vector.reciprocal_approx_fast(
                            rc[:, n * 512 : (n + 1) * 512], den)
                        nc.vector.tensor_mul(
                            aoT_sb[b][h][:, q0 + n * 512 : q0 + (n + 1) * 512],
                            av[n], rc[:, n * 512 : (n + 1) * 512])

                pending.append(den_norm)

        # ---- output projection ----
        def oproj_mm(b, tt, pss):
            # pss: 4 (psum_tile, col0) pairs covering [128, 2048]
            for h in range(HPC):
                lhsT = aoT_sb[b][h][:, tt * 128 : (tt + 1) * 128]
                for ps, oc0, w in pss:
                    nc.tensor.matmul(
                        ps,
                        lhsT,
                        woT_sb[:, h, oc0 : oc0 + w],
                        start=(h == 0),
                        stop=(h == HPC - 1),
                    )

        def oproj_gen(b, tts):
            # attention-safe filler units: only ps_half slots, DVE drains
            for tt in tts:
                o_t = o_pool.tile([128, H], BF16, tag="o", name=f"ot{b}{tt}")
                for ocp in range(2):
                    for oc in (2 * ocp, 2 * ocp + 1):
                        ps = ps_half.tile([128, 512], F32, tag="half",
                                          name=f"op{b}{tt}{oc}")
                        for h in range(HPC):
                            nc.tensor.matmul(
                                ps,
                                aoT_sb[b][h][:, tt * 128 : (tt + 1) * 128],
                                woT_sb[:, h, oc * 512 : (oc + 1) * 512],
                                start=(h == 0),
                                stop=(h == HPC - 1),
                            )
                        nc.vector.tensor_copy(
                            o_t[:, oc * 512 : (oc + 1) * 512], ps)
                    if ocp == 1:
                        row0 = b * S + tt * 128
                        eng = nc.sync if tt % 2 == 0 else nc.gpsimd
                        eng.dma_start(out=out[row0 : row0 + 128, :], in_=o_t)
                    yield

        def oproj_fast(b, tts):
            # post-attention path: 2 token-tiles in flight, drains split
            # between ScalarE (sc-slot tiles) and VectorE (av+half tiles).
            # oc-outer matmul order so each chunk's drain starts as soon as
            # its own 2 accumulation matmuls finish.
            for tt in tts:
                o_t = o_pool.tile([128, H], BF16, tag="o", name=f"ot{b}{tt}")
                if tt % 2 == 0:
                    tiles = [ps_sc.tile([128, 1024], F32, tag="sc",
                                        name=f"op{b}{tt}{k}") for k in range(2)]
                    for k in range(2):
                        for half in range(2):
                            for h in range(HPC):
                                nc.tensor.matmul(
                                    tiles[k][:, half * 512 : (half + 1) * 512],
                                    aoT_sb[b][h][:, tt * 128 : (tt + 1) * 128],
                                    woT_sb[:, h, k * 1024 + half * 512 :
                                           k * 1024 + (half + 1) * 512],
                                    start=(h == 0),
                                    stop=(h == HPC - 1),
                                )
                        nc.scalar.activation(
                            o_t[:, k * 1024 : (k + 1) * 1024], tiles[k], COPY)
                else:
                    for oc in range(4):
                        pool, tag = (ps_av, "av") if oc >= 2 else (ps_half, "half")
                        ps = pool.tile([128, 512], F32, tag=tag,
                                       name=f"op{b}{tt}{oc}")
                        for h in range(HPC):
                            nc.tensor.matmul(
                                ps,
                                aoT_sb[b][h][:, tt * 128 : (tt + 1) * 128],
                                woT_sb[:, h, oc * 512 : (oc + 1) * 512],
                                start=(h == 0),
                                stop=(h == HPC - 1),
                            )
                        nc.vector.tensor_copy(
                            o_t[:, oc * 512 : (oc + 1) * 512], ps)
                row0 = b * S + tt * 128
                eng = nc.sync if tt % 2 == 0 else nc.gpsimd
                eng.dma_start(out=out[row0 : row0 + 128, :], in_=o_t)

        # ---- main schedule ----
        qkv_half(0, 0)
        qkv_half(0, 1)
        filler_q.append(qkv_gen(1, 0))
        attention(0, 0)
        filler_q.append(qkv_gen(1, 1))
        attention(0, 1)
        drain_fillers()          # qkv(1,*) must complete before att(1,*)
        filler_q.append(oproj_gen(0, list(range(0, TC))))
        attention(1, 0, fill_every=3)
        attention(1, 1, fill_every=3)
        drain_fillers()
        flush_pending()
        oproj_fast(1, list(range(0, TC)))
        flush_pending()


def kernel(hidden_state, Wq, bq, Wk, bk, Wv, bv, Wo, bo):
    bf16 = ml_dtypes.bfloat16
    h2 = np.asarray(hidden_state, dtype=np.float32).reshape(T, H)
    hT = np.ascontiguousarray(h2.T).astype(bf16)  # [H, T], H = (c p)
    # pre-tile to [ (b blk p), c, t ] so each tile DMA is contiguous
    hT_t = np.ascontiguousarray(
        hT.reshape(FC, 128, B, 4, 512).transpose(2, 3, 1, 0, 4)
    ).reshape(8 * 128, FC, 512)

    def tile_w(w_slice_T):  # [H, HDC] -> [128, FC, HDC] partition-major
        return np.ascontiguousarray(
            w_slice_T.reshape(FC, 128, HDC).transpose(1, 0, 2)).astype(bf16)

    in_maps = []
    for c in range(N_CORES):
        r0 = c * HDC
        woT = np.asarray(Wo, np.float32)[:, r0 : r0 + HDC].T  # [HDC, H]
        in_maps.append({
            "hT": hT_t,
            "wqT": tile_w(np.asarray(Wq, np.float32)[r0 : r0 + HDC, :].T),
            "wkT": tile_w(np.asarray(Wk, np.float32)[r0 : r0 + HDC, :].T),
            "wvT": tile_w(np.asarray(Wv, np.float32)[r0 : r0 + HDC, :].T),
            "woT": np.ascontiguousarray(
                woT.reshape(HPC, 128, H).transpose(1, 0, 2)).astype(bf16),
            "bq": np.asarray(bq, np.float32)[r0 : r0 + HDC].copy(),
            "bk": np.asarray(bk, np.float32)[r0 : r0 + HDC].copy(),
            "bv": np.asarray(bv, np.float32)[r0 : r0 + HDC].reshape(1, HDC).copy(),
        })

    if "nc" not in _CACHE:
        _CACHE["nc"] = build_program()
    nc = _CACHE["nc"]
    _CACHE["in_maps"] = in_maps

    res = run_bass_kernel_spmd(nc, in_maps, core_ids=list(range(N_CORES)))
    total = np.zeros((T, H), np.float32)
    for r in res.results:
        total += np.asarray(r["out"]).astype(np.float32)
    total += np.asarray(bo, np.float32)[None, :]
    return total.reshape(B, S, H)


# revision 35
# speedup vs baseline: 1.0337x; 1.0100x over previous
"""Multi-head attention (B=2, S=2048, H=2048, NH=16) on 8 TRN2 NeuronCores.

Sharding: tensor-parallel over heads — 2 heads per core. Each core computes
q/k/v projections for its heads, per-head attention, and a partial output
projection (its heads' columns of Wo); the host sums the 8 partials.

v3: software-pipelined emission. Attention is ScalarE-bound (exp); qkv and
oproj matmul units are emitted as "fillers" between attention chunks so the
in-order TensorE queue always has ready work during exp waits.
  - PSUM: "sc" [128,1024] x2 (scores, 4 banks), "av" [128,512] x2 (AV
    accumulators, 2 banks), "half" [128,512] x2 (den / qkv chunks / filler
    oproj / fast oproj, 2 banks).
  - softmax denominator: shallow DVE add tree (depth ~5) to one [128,1024]
    bf16 sum + single deferred ones-matmul per (b,h,qh) group, flushed at
    the NEXT group's start (before its ps_av allocations — ordering matters:
    TensorE is in-order, so the den matmul must precede any matmul that
    waits on the slots it frees).
  - oproj fast path: 2 token-tiles in flight (sc slots / av+half slots);
    drains split between ScalarE ([128,1024] copies) and VectorE.
  - output rows DMA'd as [128,2048] tiles (4KB lines) on sync/gpsimd;
    hT streamed in 4-fc-chunk DMAs (sync for b=0, gpsimd for b=1);
    weights 2 chunks each, earliest-consumed first.
"""

import sys

sys.path.insert(0, "/opt/trn_rl_repo")

from contextlib import ExitStack

import ml_dtypes
import numpy as np

import concourse.bass as bass
import concourse.tile as tile
from concourse import bacc, mybir
from concourse.bass_utils import run_bass_kernel_spmd

B, S, H, NH = 2, 2048, 2048, 16
HD = H // NH          # 128
N_CORES = 8
HPC = NH // N_CORES   # heads per core = 2
HDC = HPC * HD        # head-dims per core = 256
T = B * S             # 4096 tokens
FC = H // 128         # 16 feature chunks
TC = S // 128         # 16 token tiles per batch
SHIFT = 4.0           # fixed exp shift (softmax-invariant, overflow guard)

BF16 = mybir.dt.bfloat16
F32 = mybir.dt.float32
EXP = mybir.ActivationFunctionType.Exp
COPY = mybir.ActivationFunctionType.Copy

_CACHE = {}


def build_program(out_dtype=BF16):
    nc = bacc.Bacc(
        "TRN2", target_bir_lowering=False, debug=False, num_devices=N_CORES
    )
    # all pre-tiled host-side to partition-major so DMA lines are 4-16KB
    hT = nc.dram_tensor("hT", [8 * 128, FC, 512], BF16, kind="ExternalInput").ap()
    wqT = nc.dram_tensor("wqT", [128, FC, HDC], BF16, kind="ExternalInput").ap()
    wkT = nc.dram_tensor("wkT", [128, FC, HDC], BF16, kind="ExternalInput").ap()
    wvT = nc.dram_tensor("wvT", [128, FC, HDC], BF16, kind="ExternalInput").ap()
    woT = nc.dram_tensor("woT", [128, HPC, H], BF16, kind="ExternalInput").ap()
    bq = nc.dram_tensor("bq", [HDC], F32, kind="ExternalInput").ap()
    bk = nc.dram_tensor("bk", [HDC], F32, kind="ExternalInput").ap()
    bv = nc.dram_tensor("bv", [1, HDC], F32, kind="ExternalInput").ap()
    out = nc.dram_tensor("out", [T, H], out_dtype, kind="ExternalOutput").ap()

    with tile.TileContext(nc) as tc:
        _kernel(tc, out, hT, wqT, wkT, wvT, woT, bq, bk, bv)
    nc.compile()
    return nc


def _kernel(tc, out, hT, wqT, wkT, wvT, woT, bq, bk, bv):
    nc = tc.nc
    scale = 1.0 / float(np.sqrt(HD))
    ctx = ExitStack()
    with ctx:
        singles = ctx.enter_context(tc.tile_pool(name="singles", bufs=1))
        persist = ctx.enter_context(tc.tile_pool(name="persist", bufs=1))
        ps_sc = ctx.enter_context(tc.tile_pool(name="ps_sc", bufs=2, space="PSUM"))
        ps_av = ctx.enter_context(tc.tile_pool(name="ps_av", bufs=2, space="PSUM"))
        ps_half = ctx.enter_context(tc.tile_pool(name="ps_half", bufs=2, space="PSUM"))
        ht_pool = ctx.enter_context(tc.tile_pool(name="ht", bufs=3))
        pt_pool = ctx.enter_context(tc.tile_pool(name="pt", bufs=5))
        ts_pool = ctx.enter_context(tc.tile_pool(name="ts", bufs=6))
        rc_pool = ctx.enter_context(tc.tile_pool(name="rc", bufs=2))
        o_pool = ctx.enter_context(tc.tile_pool(name="o_sb", bufs=4))

        # ---- constants ----
        ones = singles.tile([128, 128], BF16)
        nc.vector.memset(ones, 1.0)
        neg_shift = singles.tile([128, 1], F32)
        nc.vector.memset(neg_shift, -SHIFT)

        # HAM pre-warm: keep the PE busy during the input-DMA head wait so
        # the clock gate is at 8/8 when real matmuls start (~3.4us window)
        warm_ps = ps_half.tile([128, 512], F32, tag="half", name="warm")
        for i in range(40):
            nc.tensor.matmul(warm_ps[:, 0:128], ones, ones,
                             start=True, stop=True)

        # ---- weights: 2-chunk DMAs, earliest-consumed first ----
        w_sb = {}
        for name in ("v", "q", "k"):
            w_sb[name] = singles.tile(
                [128, FC, HDC], BF16, tag=f"w{name}", name=f"w{name}"
            )
        for g in range(4):
            nc.gpsimd.dma_start(
                out=w_sb["v"][:, 4 * g : 4 * g + 4, :],
                in_=wvT[:, 4 * g : 4 * g + 4, :],
            )
        for name, ap in (("q", wqT), ("k", wkT)):
            for g in range(2):
                nc.gpsimd.dma_start(
                    out=w_sb[name][:, 8 * g : 8 * g + 8, :],
                    in_=ap[:, 8 * g : 8 * g + 8, :],
                )
        woT_sb = singles.tile([128, HPC, H], BF16)
        nc.gpsimd.dma_start(out=woT_sb, in_=woT)
        bq_sb = singles.tile([128, HPC], F32)
        nc.scalar.dma_start(out=bq_sb, in_=bq.rearrange("(h p) -> p h", p=128))
        bk_sb = singles.tile([128, HPC], F32)
        nc.scalar.dma_start(out=bk_sb, in_=bk.rearrange("(h p) -> p h", p=128))
        bv2 = singles.tile([128, 2, HDC], F32)
        nc.scalar.dma_start(
            out=bv2,
            in_=bass.AP(tensor=bv.tensor, offset=bv.offset,
                        ap=[[0, 128], [0, 2], [1, HDC]]),
        )

        # ---- persistent activations ----
        qt_sb = [[persist.tile([128, S], BF16, tag=f"qt{b}{h}", name=f"qt{b}{h}")
                  for h in range(HPC)] for b in range(B)]
        kt_sb = [[persist.tile([128, S], BF16, tag=f"kt{b}{h}", name=f"kt{b}{h}")
                  for h in range(HPC)] for b in range(B)]
        v_sb = [persist.tile([128, TC, HDC], BF16, tag=f"v{b}", name=f"v{b}")
                for b in range(B)]
        aoT_sb = [[persist.tile([128, S], BF16, tag=f"ao{b}{h}", name=f"ao{b}{h}")
                   for h in range(HPC)] for b in range(B)]

        ht_tiles = {}

        def load_ht(b, blk):
            key = (b, blk)
            if key in ht_tiles:
                return ht_tiles[key]
            bi = b * 4 + blk
            t = ht_pool.tile([128, FC, 512], BF16, tag="ht", name=f"ht{b}{blk}")
            eng = nc.sync if b == 0 else nc.gpsimd
            for g in range(4):
                eng.dma_start(
                    out=t[:, 4 * g : 4 * g + 4, :],
                    in_=hT[bi * 128 : (bi + 1) * 128, 4 * g : 4 * g + 4, :],
                )
            ht_tiles[key] = t
            return t

        # ---- deferred softmax denominator ----
        pending = []

        def flush_pending():
            while pending:
                pending.pop(0)()

        # ---- filler queue (units of TensorE work to hide exp waits) ----
        filler_q = []

        def fill(n):
            while n > 0 and filler_q:
                try:
                    next(filler_q[0])
                    n -= 1
                except StopIteration:
                    filler_q.pop(0)

        def drain_fillers():
            while filler_q:
                fill(1)

        # ---- qkv projection units ----
        def v_block(b, ht_t, blk, s2):
            ps = ps_half.tile([128, 2, HDC], F32, tag="half",
                              name=f"v{b}{blk}{s2}")
            for s in range(2):
                col = (2 * s2 + s) * 128
                for fc in range(FC):
                    nc.tensor.matmul(
                        ps[:, s, :],
                        ht_t[:, fc, col : col + 128],
                        w_sb["v"][:, fc, :],
                        start=(fc == 0),
                        stop=(fc == FC - 1),
                    )
            tt0 = blk * 4 + s2 * 2
            nc.vector.tensor_add(v_sb[b][:, tt0 : tt0 + 2, :], ps, bv2)

        def qk_block(b, ht_t, blk, h, name, dst, bias):
            ps = ps_half.tile([128, 512], F32, tag="half",
                              name=f"qk{b}{blk}{h}{name}")
            for fc in range(FC):
                nc.tensor.matmul(
                    ps,
                    w_sb[name][:, fc, h * HD : (h + 1) * HD],
                    ht_t[:, fc, :],
                    start=(fc == 0),
                    stop=(fc == FC - 1),
                )
            nc.vector.tensor_scalar_add(
                dst[:, blk * 512 : (blk + 1) * 512], ps, bias[:, h : h + 1],
            )

        def qkv_gen(b, half):
            for qx in range(2):
                blk = half * 2 + qx
                ht_t = load_ht(b, blk)
                if blk + 1 < 4:
                    load_ht(b, blk + 1)
                elif b == 0:
                    load_ht(1, 0)
                v_block(b, ht_t, blk, 0)
                yield
                for h in range(HPC):
                    qk_block(b, ht_t, blk, h, "q", qt_sb[b][h], bq_sb)
                    yield
                    qk_block(b, ht_t, blk, h, "k", kt_sb[b][h], bk_sb)
                    yield
                v_block(b, ht_t, blk, 1)
                yield

        def qkv_half(b, half):
            for _ in qkv_gen(b, half):
                pass

        # ---- attention ----
        def attention(b, qh, fill_every=2):
            q0 = qh * 1024
            for h in range(HPC):
                av = [ps_av.tile([128, 512], F32, tag="av",
                                 name=f"av{b}{h}{qh}{n}") for n in range(2)]
                st = {"pt_prev": None, "t2_prev": None, "acc": None}

                def post(j, pt, b=b, h=h, qh=qh, av=av, st=st,
                         fill_every=fill_every):
                    # av matmuls + denominator-tree step for chunk j,
                    # emitted one chunk late so they never wait on exp
                    for n in range(2):
                        nc.tensor.matmul(
                            av[n],
                            v_sb[b][:, j, h * HD : (h + 1) * HD],
                            pt[:, n * 512 : (n + 1) * 512],
                            start=(j == 0),
                            stop=(j == TC - 1),
                        )
                    if j % fill_every == fill_every - 1:
                        fill(1)
                    # tree: pt pairs -> t2, t2 pairs -> p4, chain p4s
                    if st["pt_prev"] is None:
                        st["pt_prev"] = pt
                    else:
                        t2 = ts_pool.tile([128, 1024], BF16, tag="ts",
                                          name=f"t2{b}{h}{qh}{j}")
                        nc.vector.tensor_add(t2, st["pt_prev"], pt)
                        st["pt_prev"] = None
                        if st["t2_prev"] is None:
                            st["t2_prev"] = t2
                        else:
                            p4 = ts_pool.tile([128, 1024], BF16, tag="ts",
                                              name=f"p4{b}{h}{qh}{j}")
                            nc.vector.tensor_add(p4, st["t2_prev"], t2)
                            st["t2_prev"] = None
                            if st["acc"] is None:
                                st["acc"] = p4
                            else:
                                nacc = ts_pool.tile(
                                    [128, 1024], BF16, tag="ts",
                                    name=f"acc{b}{h}{qh}{j}")
                                nc.vector.tensor_add(nacc, st["acc"], p4)
                                st["acc"] = nacc

                prev = None  # (j, pt) of the previous chunk
                for tcx in range(TC):
                    ps = ps_sc.tile([128, 1024], F32, tag="sc",
                                    name=f"sc{b}{h}{qh}{tcx}")
                    lhsT = kt_sb[b][h][:, tcx * 128 : (tcx + 1) * 128]
                    for n in range(2):
                        nc.tensor.matmul(
                            ps[:, n * 512 : (n + 1) * 512],
                            lhsT,
                            qt_sb[b][h][:, q0 + n * 512 : q0 + (n + 1) * 512],
                            start=True,
                            stop=True,
                        )
                    pt = pt_pool.tile([128, 1024], BF16, tag="pt",
                                      name=f"pt{b}{h}{qh}{tcx}")
                    nc.scalar.activation(pt, ps, EXP,
                                         bias=neg_shift, scale=scale)
                    if tcx == 0:
                        # previous group's den/normalize: emitted after this
                        # group's first scores+exp (covers the DVE-chain
                        # latency) but before the first av matmul reuses
                        # the ps_av slots it still holds.
                        flush_pending()
                    if prev is not None:
                        post(*prev)
                    prev = (tcx, pt)
                post(*prev)
                acc = st["acc"]

                def den_norm(b=b, h=h, q0=q0, av=av, acc=acc):
                    rc = rc_pool.tile([128, 1024], F32, tag="rc",
                                      name=f"rc{b}{h}{q0}")
                    for n in range(2):
                        den = ps_half.tile([128, 512], F32, tag="half",
                                           name=f"den{b}{h}{q0}{n}")
                        nc.tensor.matmul(
                            den, ones, acc[:, n * 512 : (n + 1) * 512],
                            start=True, stop=True,
                        )
                        nc.vector.reciprocal_approx_fast(
                            rc[:, n * 512 : (n + 1) * 512], den)
                        nc.vector.tensor_mul(
                            aoT_sb[b][h][:, q0 + n * 512 : q0 + (n + 1) * 512],
                            av[n], rc[:, n * 512 : (n + 1) * 512])

                pending.append(den_norm)

        # ---- output projection ----
        def oproj_mm(b, tt, pss):
            # pss: 4 (psum_tile, col0) pairs covering [128, 2048]
            for h in range(HPC):
                lhsT = aoT_sb[b][h][:, tt * 128 : (tt + 1) * 128]
                for ps, oc0, w in pss:
                    nc.tensor.matmul(
                        ps,
                        lhsT,
                        woT_sb[:, h, oc0 : oc0 + w],
                        start=(h == 0),
                        stop=(h == HPC - 1),
                    )

        att_done = [False]

        def oproj_gen(b, tts):
            # filler units: only ps_half slots. During attention the drains
            # go to DVE (ScalarE is the attention bottleneck); once
            # attention is over, alternate ScalarE/DVE so the leftover
            # units drain at TensorE pace.
            for tt in tts:
                o_t = o_pool.tile([128, H], BF16, tag="o", name=f"ot{b}{tt}")
                for ocp in range(2):
                    for oc in (2 * ocp, 2 * ocp + 1):
                        ps = ps_half.tile([128, 512], F32, tag="half",
                                          name=f"op{b}{tt}{oc}")
                        for h in range(HPC):
                            nc.tensor.matmul(
                                ps,
                                aoT_sb[b][h][:, tt * 128 : (tt + 1) * 128],
                                woT_sb[:, h, oc * 512 : (oc + 1) * 512],
                                start=(h == 0),
                                stop=(h == HPC - 1),
                            )
                        if att_done[0] and oc % 2 == 0:
                            nc.scalar.activation(
                                o_t[:, oc * 512 : (oc + 1) * 512], ps, COPY)
                        else:
                            nc.vector.tensor_copy(
                                o_t[:, oc * 512 : (oc + 1) * 512], ps)
                    if ocp == 1:
                        row0 = b * S + tt * 128
                        eng = nc.sync if tt % 2 == 0 else nc.gpsimd
                        eng.dma_start(out=out[row0 : row0 + 128, :], in_=o_t)
                    yield

        def oproj_fast(b, tts):
            # post-attention path: 2 token-tiles in flight, drains split
            # between ScalarE (sc-slot tiles) and VectorE (av+half tiles).
            # oc-outer matmul order so each chunk's drain starts as soon as
            # its own 2 accumulation matmuls finish.
            for tt in tts:
                o_t = o_pool.tile([128, H], BF16, tag="o", name=f"ot{b}{tt}")
                if tt % 2 == 0:
                    tiles = [ps_sc.tile([128, 1024], F32, tag="sc",
                                        name=f"op{b}{tt}{k}") for k in range(2)]
                    for k in range(2):
                        for half in range(2):
                            for h in range(HPC):
                                nc.tensor.matmul(
                                    tiles[k][:, half * 512 : (half + 1) * 512],
                                    aoT_sb[b][h][:, tt * 128 : (tt + 1) * 128],
                                    woT_sb[:, h, k * 1024 + half * 512 :
                                           k * 1024 + (half + 1) * 512],
                                    start=(h == 0),
                                    stop=(h == HPC - 1),
                                )
                        nc.scalar.activation(
                            o_t[:, k * 1024 : (k + 1) * 1024], tiles[k], COPY)
                else:
                    for oc in range(4):
                        pool, tag = (ps_av, "av") if oc >= 2 else (ps_half, "half")
                        ps = pool.tile([128, 512], F32, tag=tag,
                                       name=f"op{b}{tt}{oc}")
                        for h in range(HPC):
                            nc.tensor.matmul(
                                ps,
                                aoT_sb[b][h][:, tt * 128 : (tt + 1) * 128],
                                woT_sb[:, h, oc * 512 : (oc + 1) * 512],
                                start=(h == 0),
                                stop=(h == HPC - 1),
                            )
                        nc.vector.tensor_copy(
                            o_t[:, oc * 512 : (oc + 1) * 512], ps)
                row0 = b * S + tt * 128
                eng = (nc.sync, nc.gpsimd, nc.scalar)[tt % 3]
                eng.dma_start(out=out[row0 : row0 + 128, :], in_=o_t)

        # ---- main schedule ----
        qkv_half(0, 0)
        qkv_half(0, 1)
        filler_q.append(qkv_gen(1, 0))
        attention(0, 0)
        filler_q.append(qkv_gen(1, 1))
        attention(0, 1)
        drain_fillers()          # qkv(1,*) must complete before att(1,*)
        filler_q.append(oproj_gen(0, list(range(0, TC))))
        attention(1, 0, fill_every=3)
        attention(1, 1, fill_every=3)
        att_done[0] = True
        drain_fillers()
        flush_pending()
        oproj_fast(1, list(range(0, TC)))
        flush_pending()


def kernel(hidden_state, Wq, bq, Wk, bk, Wv, bv, Wo, bo):
    bf16 = ml_dtypes.bfloat16
    h2 = np.asarray(hidden_state, dtype=np.float32).reshape(T, H)
    hT = np.ascontiguousarray(h2.T).astype(bf16)  # [H, T], H = (c p)
    # pre-tile to [ (b blk p), c, t ] so each tile DMA is contiguous
    hT_t = np.ascontiguousarray(
        hT.reshape(FC, 128, B, 4, 512).transpose(2, 3, 1, 0, 4)
    ).reshape(8 * 128, FC, 512)

    def tile_w(w_slice_T):  # [H, HDC] -> [128, FC, HDC] partition-major
        return np.ascontiguousarray(
            w_slice_T.reshape(FC, 128, HDC).transpose(1, 0, 2)).astype(bf16)

    in_maps = []
    for c in range(N_CORES):
        r0 = c * HDC
        woT = np.asarray(Wo, np.float32)[:, r0 : r0 + HDC].T  # [HDC, H]
        in_maps.append({
            "hT": hT_t,
            "wqT": tile_w(np.asarray(Wq, np.float32)[r0 : r0 + HDC, :].T),
            "wkT": tile_w(np.asarray(Wk, np.float32)[r0 : r0 + HDC, :].T),
            "wvT": tile_w(np.asarray(Wv, np.float32)[r0 : r0 + HDC, :].T),
            "woT": np.ascontiguousarray(
                woT.reshape(HPC, 128, H).transpose(1, 0, 2)).astype(bf16),
            "bq": np.asarray(bq, np.float32)[r0 : r0 + HDC].copy(),
            "bk": np.asarray(bk, np.float32)[r0 : r0 + HDC].copy(),
            "bv": np.asarray(bv, np.float32)[r0 : r0 + HDC].reshape(1, HDC).copy(),
        })

    if "nc" not in _CACHE:
        _CACHE["nc"] = build_program()
    nc = _CACHE["nc"]
    _CACHE["in_maps"] = in_maps

    res = run_bass_kernel_spmd(nc, in_maps, core_ids=list(range(N_CORES)))
    total = np.zeros((T, H), np.float32)
    for r in res.results:
        total += np.asarray(r["out"]).astype(np.float32)
    total += np.asarray(bo, np.float32)[None, :]
    return total.reshape(B, S, H)
